# revision 2
# baseline (speedup 1.0000x reference)
"""BiLSTM (T=2048, B=32, I=H=256) Bass kernel for 8 NeuronCores, v2.

v2 vs v1: UNROLL=2 (5x smaller NEFF), one merged sigmoid per dir-step,
xp deposited into the gate PSUM by the Pool engine (frees 2 PE pairs per
step), cell state kept in f16 (halves SBUF + lets h/c share one packed
f16 output tensor with zero padding rows), constants packed into two
input tensors, phase-A bias moves rebalanced onto DVE/Pool so ACT only
runs the latency-critical sigmoid/tanh.

Sharding (data-parallel per the hint): batch is split 8 ways; every core
runs BOTH directions for its 4 batch lanes as two independent chains,
interleaved op-by-op so each chain's cross-engine latency is hidden by
the other chain's work. Backward direction consumes x flipped along time
AND batch (faithful to torch.flip(input_, [0,1])).

On-chip layout is fully transposed ([H partitions, lanes free]); gate
rows are permuted to [f,g,i,o] with g pre-scaled by 2 so one Sigmoid
covers all four gates (tanh(g) = 2*sigmoid(2g) - 1).

Length masking is exact and handled on the host: post-length steps
compute garbage confined to their own lanes; the output tail t >= len is
overwritten host-side with the frozen value at len-1.
"""

import sys

import numpy as np

# ---- problem constants (hardcoded per contract) ----
T, B, I, H = 2048, 32, 256, 256
NCORES = 8
ND = 2            # directions per core
BL = B // NCORES  # 4 batch lanes per core per direction
B2 = 2 * BL       # (H-tile, lane) free width of h/c state
G = 8             # 4H/128 gate row tiles, order [f0,f1,g0,g1,i0,i1,o0,o1]
KT = 2            # H/128 contraction tiles
TC = 128          # scan chunk length (steps per For_i iteration)
DTYPE = "f16"     # weights/x/h/c on-chip + packed output dtype
XP_DEPOSIT = "pe"  # "pe" | "dve": engine seeding gate PSUM (Pool can't)
UNROLL = 2

_CACHE = {}


def _import_bass():
    try:
        import concourse.bass  # noqa: F401
    except ImportError:
        sys.path.insert(0, "/opt/trn_rl_repo")


def build_program(t_total=T, tc=TC, dtype=DTYPE, xp_dep=XP_DEPOSIT):
    """Build the SPMD Bass program (identical on all cores)."""
    _import_bass()
    import concourse.bass as bass
    import concourse.mybir as mybir
    from concourse import bacc
    from concourse.tile import TileContext

    ds = bass.ds
    f32 = mybir.dt.float32
    dt_w = {"f32": f32, "bf16": mybir.dt.bfloat16,
            "f16": mybir.dt.float16}[dtype]
    AF = mybir.ActivationFunctionType
    OP = mybir.AluOpType

    n_chunks = t_total // tc
    assert t_total % tc == 0

    nc = bacc.Bacc("TRN2", target_bir_lowering=False, debug=False,
                   num_devices=NCORES)

    # DRAM I/O. Chunk row stride is ND*KT*128 = 512 for xarr; hc_out packs
    # [h_d0, h_d1, c_d0, c_d1] x 128 rows per chunk -> same 512-row stride
    # so one For_i loop var addresses everything. +1 zero-padded chunk on
    # xarr so the steady-state prefetch of chunk k+1 stays in-bounds.
    xarr = nc.dram_tensor("xarr", [(n_chunks + 1) * ND * KT * 128, tc * BL],
                          dt_w, kind="ExternalInput")
    # wpack rows: whhT (ND*KT*G*128) | wihT (ND*KT*G*128) | ident (128)
    WROWS = ND * KT * G * 128
    wpack = nc.dram_tensor("wpack", [2 * WROWS + 128, 128], dt_w,
                           kind="ExternalInput")
    # fpack cols: biasT (ND*G) | hc0T (ND*2*B2)
    fpack = nc.dram_tensor("fpack", [128, ND * G + ND * 2 * B2], f32,
                           kind="ExternalInput")
    hc_out = nc.dram_tensor("hc_out", [n_chunks * 4 * 128, tc * B2], dt_w,
                            kind="ExternalOutput")

    from contextlib import ExitStack
    with TileContext(nc) as tcx, ExitStack() as stk:
        wpool = stk.enter_context(tcx.tile_pool(name="weights", bufs=1))
        spool = stk.enter_context(tcx.tile_pool(name="state", bufs=1))
        xpool = stk.enter_context(tcx.tile_pool(name="xdata", bufs=1))
        tpool = stk.enter_context(tcx.tile_pool(name="temps", bufs=3))
        pgpool = stk.enter_context(tcx.tile_pool(name="psg", bufs=2,
                                                 space="PSUM"))

        whh_sb = wpool.tile([128, ND * KT * G * 128], dt_w)
        wih_sb = wpool.tile([128, ND * KT * G * 128], dt_w)
        bias_sb = wpool.tile([128, ND * G], f32)
        hc0_sb = wpool.tile([128, ND * 2 * B2], f32)
        ident_sb = wpool.tile([128, 128], dt_w)
        HB = (tc + 1) * B2  # per-direction history block
        # double-buffered per chunk so the output DMA of chunk k drains
        # while the scan of chunk k+1 writes the other buffer
        h_hist = [spool.tile([128, ND * HB], dt_w, name=f"hh{b}")
                  for b in range(2)]
        c_hist = [spool.tile([128, ND * HB], dt_w, name=f"ch{b}")
                  for b in range(2)]
        xp = [[xpool.tile([128, G * tc * BL], dt_w, name=f"xp{b}{d}")
               for d in range(ND)] for b in range(2)]
        xin = [xpool.tile([128, ND * KT * tc * BL], dt_w, name=f"xin{b}")
               for b in range(2)]

        def w_sl(sb, d, ki, j):
            off = ((d * KT + ki) * G + j) * 128
            return sb[:, off:off + 128]

        def h_sl(b, d, slot, ki=0, w=None):
            off = d * HB + slot * B2 + ki * BL
            return h_hist[b][:, off:off + (w if w is not None else B2)]

        def c_sl(b, d, slot):
            off = d * HB + slot * B2
            return c_hist[b][:, off:off + B2]

        # --- load constants ---
        nc.sync.dma_start(
            out=whh_sb[:].rearrange("p (a m) -> p a m", m=128),
            in_=wpack.ap()[0:WROWS, :].rearrange("(a p) m -> p a m", p=128))
        nc.sync.dma_start(
            out=wih_sb[:].rearrange("p (a m) -> p a m", m=128),
            in_=wpack.ap()[WROWS:2 * WROWS, :]
                .rearrange("(a p) m -> p a m", p=128))
        nc.sync.dma_start(out=ident_sb[:],
                          in_=wpack.ap()[2 * WROWS:2 * WROWS + 128, :])
        nc.sync.dma_start(out=bias_sb[:], in_=fpack.ap()[:, 0:ND * G])
        nc.sync.dma_start(out=hc0_sb[:], in_=fpack.ap()[:, ND * G:])
        for d in range(ND):
            nc.vector.tensor_copy(h_sl(0, d, 0),
                                  hc0_sb[:, (2 * d) * B2:(2 * d + 1) * B2])
            nc.vector.tensor_copy(c_sl(0, d, 0),
                                  hc0_sb[:, (2 * d + 1) * B2:(2 * d + 2) * B2])

        assert tc * BL == 512

        PA_SUB = 4  # split each unit's PSUM->SBUF bias move into quarters

        def emit_pa_mm(unit, xin_t):
            # one phase-A matmul pair: psa = Wih[j] @ x.T
            d, j = divmod(unit, G)
            ps = pgpool.tile([128, tc * BL], f32, tag=f"pa{d}", name="psa")
            for ki in range(KT):
                a = (d * KT + ki)
                nc.tensor.matmul(
                    ps[:], w_sl(wih_sb, d, ki, j),
                    xin_t[:, a * tc * BL:(a + 1) * tc * BL],
                    start=(ki == 0), stop=(ki == KT - 1))
            return ps

        def emit_pa_bias(unit, ps, q, xp_t):
            # quarter-width bias + PSUM->SBUF move, all on DVE (ACT is
            # reserved for the latency-critical sigmoid/tanh chain; Pool
            # can't access PSUM)
            d, j = divmod(unit, G)
            w = tc * BL // PA_SUB
            bcol = bias_sb[:, d * G + j:d * G + j + 1]
            src = ps[:, q * w:(q + 1) * w]
            dst = xp_t[d][:, j * tc * BL + q * w:j * tc * BL + (q + 1) * w]
            nc.vector.tensor_scalar(dst, src, bcol, None, OP.add)

        def prefetch_x(buf, kbase):
            nc.sync.dma_start(
                out=xin[buf][:].rearrange("p (a n) -> p a n", a=ND * KT),
                in_=xarr.ap()[ds(kbase, ND * KT * 128), :]
                    .rearrange("(a p) n -> p a n", p=128))

        def chunk_body(kbase, cur):
            # scan chunk at kbase using xp[cur]; prefetch + phase A for the
            # next chunk interleaved into the scan.
            nxt = 1 - cur
            prefetch_x(nxt, kbase + ND * KT * 128)
            pa_ps = {}
            for tl in range(tc):
                if 8 <= tl and (tl - 8) // 7 < ND * G:
                    u_, ph = divmod(tl - 8, 7)
                    if ph == 0:
                        pa_ps[u_] = emit_pa_mm(u_, xin[nxt])
                    elif ph <= PA_SUB:
                        emit_pa_bias(u_, pa_ps[u_], ph - 1, xp[nxt])
                if tl == tc // 2 + 1:
                    # first-half output flush (h rows kbase+0..255, c rows
                    # kbase+256..511) drains during the scan
                    half = (tc // 2) * B2
                    nc.sync.dma_start(
                        out=hc_out.ap()[ds(kbase, ND * 128), 0:half]
                            .rearrange("(a p) n -> p a n", p=128),
                        in_=h_hist[cur][:].rearrange(
                            "p (a n) -> p a n", a=ND)[:, :, B2:B2 + half])
                    nc.sync.dma_start(
                        out=hc_out.ap()[ds(kbase + ND * 128, ND * 128),
                                        0:half]
                            .rearrange("(a p) n -> p a n", p=128),
                        in_=c_hist[cur][:].rearrange(
                            "p (a n) -> p a n", a=ND)[:, :, B2:B2 + half])
                psg = [None, None]
                xpv = [xp[cur][d][:].rearrange("p (g t l) -> p g t l",
                                               g=G, l=BL)[:, :, tl, :]
                       for d in range(ND)]
                for d in range(ND):
                    ps = pgpool.tile([128, G * BL], f32, tag=f"g{d}",
                                     name="psg")
                    psg[d] = ps
                    # deposit xp into the gate bank ahead of the h MMs
                    if xp_dep == "dve":
                        nc.vector.tensor_copy(
                            ps[:].rearrange("p (g l) -> p g l", l=BL),
                            xpv[d])
                    else:
                        nc.tensor.matmul(
                            ps[:].rearrange("p (g l) -> p g l", l=BL),
                            ident_sb[:], xpv[d],
                            start=True, stop=False,
                            skip_group_check=True)
                    for j in range(G):
                        for ki in range(KT):
                            nc.tensor.matmul(
                                ps[:, j * BL:(j + 1) * BL],
                                w_sl(whh_sb, d, ki, j),
                                h_sl(cur, d, tl, ki, BL),
                                start=False,
                                stop=(ki == KT - 1 and j == G - 1),
                                skip_group_check=True)
                sig, amr, cf, u, tcl = [], [], [], [], []
                for d in range(ND):
                    sig.append(tpool.tile([128, G * BL], f32, tag=f"sg{d}",
                                          name="sig"))
                    amr.append(tpool.tile([128, 1], f32, tag=f"am{d}",
                                          name="amr"))
                    cf.append(tpool.tile([128, B2], f32, tag=f"cf{d}",
                                         name="cf"))
                    u.append(tpool.tile([128, B2], f32, tag=f"u{d}",
                                        name="u"))
                    tcl.append(tpool.tile([128, B2], f32, tag=f"tc{d}",
                                          name="tcl"))
                for d in range(ND):  # one sigmoid across all 8 gate tiles
                    nc.scalar.activation(sig[d][:], psg[d][:], AF.Sigmoid)
                for d in range(ND):  # cf = sig(f) * c_prev   [f tiles 0,1]
                    nc.gpsimd.tensor_mul(cf[d][:], sig[d][:, 0:2 * BL],
                                         c_sl(cur, d, tl))
                for d in range(ND):  # u = (2*sig(2g)-1) * sig(i) fused
                    nc.vector.affine_mul_reduce(
                        u[d][:], amr[d][:], sig[d][:, 2 * BL:4 * BL],
                        sig[d][:, 4 * BL:6 * BL], 2.0, -1.0)
                for d in range(ND):  # c_new (f16 history write)
                    nc.gpsimd.tensor_add(c_sl(cur, d, tl + 1),
                                         cf[d][:], u[d][:])
                for d in range(ND):
                    nc.scalar.activation(tcl[d][:], c_sl(cur, d, tl + 1),
                                         AF.Tanh)
                for d in range(ND):  # h = sig(o) * tanh(c)   [o tiles 6,7]
                    nc.vector.tensor_mul(h_sl(cur, d, tl + 1),
                                         sig[d][:, 6 * BL:8 * BL], tcl[d][:])
            # flush second-half outputs; carry state into the OTHER history
            # buffer so the drain never blocks the next chunk's scan
            half = (tc // 2) * B2
            nc.sync.dma_start(
                out=hc_out.ap()[ds(kbase, ND * 128), half:]
                    .rearrange("(a p) n -> p a n", p=128),
                in_=h_hist[cur][:].rearrange("p (a n) -> p a n",
                                             a=ND)[:, :, B2 + half:])
            nc.sync.dma_start(
                out=hc_out.ap()[ds(kbase + ND * 128, ND * 128), half:]
                    .rearrange("(a p) n -> p a n", p=128),
                in_=c_hist[cur][:].rearrange("p (a n) -> p a n",
                                             a=ND)[:, :, B2 + half:])
            for d in range(ND):
                nc.gpsimd.tensor_copy(h_sl(1 - cur, d, 0), h_sl(cur, d, tc))
                nc.gpsimd.tensor_copy(c_sl(1 - cur, d, 0), c_sl(cur, d, tc))

        # prologue: fetch + phase A for chunk 0 into buffer 0
        prefetch_x(0, 0)
        for unit in range(ND * G):
            ps = emit_pa_mm(unit, xin[0])
            for q in range(PA_SUB):
                emit_pa_bias(unit, ps, q, xp[0])

        assert n_chunks % UNROLL == 0
        CR = ND * KT * 128
        import concourse.mybir as _mb
        with tcx.For_i(0, n_chunks * CR, UNROLL * CR,
                       hint_engines=(_mb.EngineType.PE,
                                     _mb.EngineType.Activation,
                                     _mb.EngineType.DVE,
                                     _mb.EngineType.Pool)) as kbase:
            for uu in range(UNROLL):
                chunk_body(kbase + uu * CR, uu % 2)

    nc.compile()
    return nc


# ---------------- host-side data marshalling ----------------

def _perm_scale_rows(w):
    """Reorder gate rows [i,f,g,o] -> [f,g,i,o], scale g rows by 2."""
    return np.concatenate(
        [w[256:512], 2.0 * w[512:768], w[0:256], w[768:1024]], 0)


def _np_dt(dtype):
    import ml_dtypes
    return {"f32": np.float32, "bf16": ml_dtypes.bfloat16,
            "f16": np.float16}[dtype]


def prep_inputs(x, length, h0, c0, Wih_f, Whh_f, bih_f, bhh_f,
                Wih_b, Whh_b, bih_b, bhh_b, t_total=T, tc=TC, dtype=DTYPE):
    """Build per-core input dicts."""
    n_chunks = t_total // tc
    dt = _np_dt(dtype)
    x = np.asarray(x, np.float32)
    x_b = x[::-1, ::-1, :]

    wihP = {0: _perm_scale_rows(np.asarray(Wih_f)),
            1: _perm_scale_rows(np.asarray(Wih_b))}
    whhP = {0: _perm_scale_rows(np.asarray(Whh_f)),
            1: _perm_scale_rows(np.asarray(Whh_b))}
    biasP = {0: _perm_scale_rows(
                 (np.asarray(bih_f) + np.asarray(bhh_f))[:, None]),
             1: _perm_scale_rows(
                 (np.asarray(bih_b) + np.asarray(bhh_b))[:, None])}

    def wtiles(w):
        out = np.empty((ND * KT * G * 128, 128), dt)
        for d in range(ND):
            wT = w[d].T.astype(dt)
            for ki in range(KT):
                for j in range(G):
                    off = ((d * KT + ki) * G + j) * 128
                    out[off:off + 128] = wT[ki * 128:(ki + 1) * 128,
                                            j * 128:(j + 1) * 128]
        return out

    WROWS = ND * KT * G * 128
    wpack = np.empty((2 * WROWS + 128, 128), dt)
    wpack[0:WROWS] = wtiles(whhP)
    wpack[WROWS:2 * WROWS] = wtiles(wihP)
    wpack[2 * WROWS:] = np.eye(128, dtype=dt)

    biasT = np.zeros((128, ND * G), np.float32)
    for d in range(ND):
        for j in range(G):
            biasT[:, d * G + j] = biasP[d][j * 128:(j + 1) * 128, 0]

    h0 = np.asarray(h0, np.float32)
    c0 = np.asarray(c0, np.float32)

    in_maps = []
    for core in range(NCORES):
        sl = slice(core * BL, (core + 1) * BL)
        xarr = np.zeros(((n_chunks + 1) * ND * KT * 128, tc * BL), dt)
        for d, xd in ((0, x), (1, x_b)):
            xs = xd[:t_total, sl, :]
            xT = np.ascontiguousarray(xs.transpose(0, 2, 1)).astype(dt)
            for k in range(n_chunks):
                for ki in range(KT):
                    roff = (k * ND * KT + d * KT + ki) * 128
                    blk = xT[k * tc:(k + 1) * tc,
                             ki * 128:(ki + 1) * 128, :]
                    xarr[roff:roff + 128] = (
                        blk.transpose(1, 0, 2).reshape(128, tc * BL))
        hc0T = np.zeros((128, ND * 2 * B2), np.float32)
        for d in range(ND):
            for s, st in ((0, h0), (1, c0)):
                stT = st[sl].T
                for ki in range(KT):
                    off = (2 * d + s) * B2 + ki * BL
                    hc0T[:, off:off + BL] = stT[ki * 128:(ki + 1) * 128, :]
        fpack = np.concatenate([biasT, hc0T], axis=1)
        in_maps.append({"xarr": xarr, "wpack": wpack, "fpack": fpack})
    return in_maps


def assemble_outputs(results, length, t_total=T, tc=TC):
    """results: per-core {'hc_out'}. Returns (output, cell)."""
    n_chunks = t_total // tc
    length = np.asarray(length)
    out_h = np.empty((t_total, 2 * B, H), np.float32)
    out_c = np.empty((t_total, 2 * B, H), np.float32)
    for core in range(NCORES):
        sl = slice(core * BL, (core + 1) * BL)
        arr = results[core]["hc_out"]
        v = arr.astype(np.float32).reshape(n_chunks, 2, ND, 128, tc, 2, BL)
        # [k, s(h/c), d, p, tl, ki, l] -> [s, d, (k tl), l, (ki p)]
        v = v.transpose(1, 2, 0, 4, 6, 5, 3).reshape(2, ND, t_total, BL, H)
        for s, out in ((0, out_h), (1, out_c)):
            for d in range(ND):
                col0 = d * B + sl.start
                out[:, col0:col0 + BL, :] = v[s, d]
    for b in range(B):
        ln = int(length[b])
        if ln < t_total:
            out_h[ln:, b] = out_h[ln - 1, b]
            out_c[ln:, b] = out_c[ln - 1, b]
            out_h[ln:, B + b] = out_h[ln - 1, B + b]
            out_c[ln:, B + b] = out_c[ln - 1, B + b]
    return out_h, out_c


def kernel(**inputs):
    _import_bass()
    from concourse.bass_utils import run_bass_kernel_spmd
    key = (T, TC, DTYPE)
    if key not in _CACHE:
        _CACHE[key] = build_program(T, TC)
    nc = _CACHE[key]
    in_maps = prep_inputs(**inputs)
    res = run_bass_kernel_spmd(nc, in_maps, list(range(NCORES)))
    return assemble_outputs(res.results, inputs["length"])


# revision 4
# speedup vs baseline: 1.0005x; 1.0005x over previous
"""BiLSTM (T=2048, B=32, I=H=256) Bass kernel for 8 NeuronCores, v2.

v2 vs v1: UNROLL=2 (5x smaller NEFF), one merged sigmoid per dir-step,
xp deposited into the gate PSUM by the Pool engine (frees 2 PE pairs per
step), cell state kept in f16 (halves SBUF + lets h/c share one packed
f16 output tensor with zero padding rows), constants packed into two
input tensors, phase-A bias moves rebalanced onto DVE/Pool so ACT only
runs the latency-critical sigmoid/tanh.

Sharding (data-parallel per the hint): batch is split 8 ways; every core
runs BOTH directions for its 4 batch lanes as two independent chains,
interleaved op-by-op so each chain's cross-engine latency is hidden by
the other chain's work. Backward direction consumes x flipped along time
AND batch (faithful to torch.flip(input_, [0,1])).

On-chip layout is fully transposed ([H partitions, lanes free]); gate
rows are permuted to [f,g,i,o] with g pre-scaled by 2 so one Sigmoid
covers all four gates (tanh(g) = 2*sigmoid(2g) - 1).

Length masking is exact and handled on the host: post-length steps
compute garbage confined to their own lanes; the output tail t >= len is
overwritten host-side with the frozen value at len-1.
"""

import sys

import numpy as np

# ---- problem constants (hardcoded per contract) ----
T, B, I, H = 2048, 32, 256, 256
NCORES = 8
ND = 2            # directions per core
BL = B // NCORES  # 4 batch lanes per core per direction
B2 = 2 * BL       # (H-tile, lane) free width of h/c state
G = 8             # 4H/128 gate row tiles, order [f0,f1,g0,g1,i0,i1,o0,o1]
KT = 2            # H/128 contraction tiles
TC = 64      # scan chunk length (steps per For_i iteration)
DTYPE = "f16"     # weights/x/h/c on-chip + packed output dtype
XP_DEPOSIT = "pe"  # "pe" | "dve": engine seeding gate PSUM (Pool can't)
UNROLL = 2

_CACHE = {}


def _import_bass():
    try:
        import concourse.bass  # noqa: F401
    except ImportError:
        sys.path.insert(0, "/opt/trn_rl_repo")


def build_program(t_total=T, tc=TC, dtype=DTYPE, xp_dep=XP_DEPOSIT):
    """Build the SPMD Bass program (identical on all cores)."""
    _import_bass()
    import concourse.bass as bass
    import concourse.mybir as mybir
    from concourse import bacc
    from concourse.tile import TileContext

    ds = bass.ds
    f32 = mybir.dt.float32
    dt_w = {"f32": f32, "bf16": mybir.dt.bfloat16,
            "f16": mybir.dt.float16}[dtype]
    AF = mybir.ActivationFunctionType
    OP = mybir.AluOpType

    n_chunks = t_total // tc
    assert t_total % tc == 0

    nc = bacc.Bacc("TRN2", target_bir_lowering=False, debug=False,
                   num_devices=NCORES)

    # DRAM I/O. Chunk row stride is ND*KT*128 = 512 for xarr; hc_out packs
    # [h_d0, h_d1, c_d0, c_d1] x 128 rows per chunk -> same 512-row stride
    # so one For_i loop var addresses everything. +1 zero-padded chunk on
    # xarr so the steady-state prefetch of chunk k+1 stays in-bounds.
    xarr = nc.dram_tensor("xarr", [(n_chunks + 1) * ND * KT * 128, tc * BL],
                          dt_w, kind="ExternalInput")
    # wpack rows: whhT (ND*KT*G*128) | wihT (ND*KT*G*128) | ident (128)
    WROWS = ND * KT * G * 128
    wpack = nc.dram_tensor("wpack", [2 * WROWS + 128, 128], dt_w,
                           kind="ExternalInput")
    # fpack cols: biasT (ND*G) | hc0T (ND*2*B2)
    fpack = nc.dram_tensor("fpack", [128, ND * G + ND * 2 * B2], f32,
                           kind="ExternalInput")
    hc_out = nc.dram_tensor("hc_out", [n_chunks * 4 * 128, tc * B2], dt_w,
                            kind="ExternalOutput")

    from contextlib import ExitStack
    with TileContext(nc) as tcx, ExitStack() as stk:
        wpool = stk.enter_context(tcx.tile_pool(name="weights", bufs=1))
        spool = stk.enter_context(tcx.tile_pool(name="state", bufs=1))
        xpool = stk.enter_context(tcx.tile_pool(name="xdata", bufs=1))
        tpool = stk.enter_context(tcx.tile_pool(name="temps", bufs=3))
        pgpool = stk.enter_context(tcx.tile_pool(name="psg", bufs=2,
                                                 space="PSUM"))

        whh_sb = wpool.tile([128, ND * KT * G * 128], dt_w)
        wih_sb = wpool.tile([128, ND * KT * G * 128], dt_w)
        bias_sb = wpool.tile([128, ND * G], f32)
        hc0_sb = wpool.tile([128, ND * 2 * B2], f32)
        ident_sb = wpool.tile([128, 128], dt_w)
        HB = (tc + 1) * B2  # per-direction history block
        # double-buffered per chunk so the output DMA of chunk k drains
        # while the scan of chunk k+1 writes the other buffer
        h_hist = [spool.tile([128, ND * HB], dt_w, name=f"hh{b}")
                  for b in range(2)]
        c_hist = [spool.tile([128, ND * HB], dt_w, name=f"ch{b}")
                  for b in range(2)]
        xp = [[xpool.tile([128, G * tc * BL], dt_w, name=f"xp{b}{d}")
               for d in range(ND)] for b in range(2)]
        xin = [xpool.tile([128, ND * KT * tc * BL], dt_w, name=f"xin{b}")
               for b in range(2)]

        def w_sl(sb, d, ki, j):
            off = ((d * KT + ki) * G + j) * 128
            return sb[:, off:off + 128]

        def h_sl(b, d, slot, ki=0, w=None):
            off = d * HB + slot * B2 + ki * BL
            return h_hist[b][:, off:off + (w if w is not None else B2)]

        def c_sl(b, d, slot):
            off = d * HB + slot * B2
            return c_hist[b][:, off:off + B2]

        # --- load constants ---
        nc.sync.dma_start(
            out=whh_sb[:].rearrange("p (a m) -> p a m", m=128),
            in_=wpack.ap()[0:WROWS, :].rearrange("(a p) m -> p a m", p=128))
        nc.sync.dma_start(
            out=wih_sb[:].rearrange("p (a m) -> p a m", m=128),
            in_=wpack.ap()[WROWS:2 * WROWS, :]
                .rearrange("(a p) m -> p a m", p=128))
        nc.sync.dma_start(out=ident_sb[:],
                          in_=wpack.ap()[2 * WROWS:2 * WROWS + 128, :])
        nc.sync.dma_start(out=bias_sb[:], in_=fpack.ap()[:, 0:ND * G])
        nc.sync.dma_start(out=hc0_sb[:], in_=fpack.ap()[:, ND * G:])
        for d in range(ND):
            nc.vector.tensor_copy(h_sl(0, d, 0),
                                  hc0_sb[:, (2 * d) * B2:(2 * d + 1) * B2])
            nc.vector.tensor_copy(c_sl(0, d, 0),
                                  hc0_sb[:, (2 * d + 1) * B2:(2 * d + 2) * B2])

        assert tc * BL in (256, 512)

        # phase-A pacing: one unit every PA_SP steps, bias move split into
        # PA_SUB pieces (denser at tc=64 so all 16 units fit in a chunk)
        PA_SUB = 4 if tc >= 128 else 2
        PA_SP = 7 if tc >= 128 else 3

        def emit_pa_mm(unit, xin_t):
            # one phase-A matmul pair: psa = Wih[j] @ x.T
            d, j = divmod(unit, G)
            ps = pgpool.tile([128, tc * BL], f32, tag=f"pa{d}", name="psa")
            for ki in range(KT):
                a = (d * KT + ki)
                nc.tensor.matmul(
                    ps[:], w_sl(wih_sb, d, ki, j),
                    xin_t[:, a * tc * BL:(a + 1) * tc * BL],
                    start=(ki == 0), stop=(ki == KT - 1))
            return ps

        def emit_pa_bias(unit, ps, q, xp_t):
            # quarter-width bias + PSUM->SBUF move, all on DVE (ACT is
            # reserved for the latency-critical sigmoid/tanh chain; Pool
            # can't access PSUM)
            d, j = divmod(unit, G)
            w = tc * BL // PA_SUB
            bcol = bias_sb[:, d * G + j:d * G + j + 1]
            src = ps[:, q * w:(q + 1) * w]
            dst = xp_t[d][:, j * tc * BL + q * w:j * tc * BL + (q + 1) * w]
            nc.vector.tensor_scalar(dst, src, bcol, None, OP.add)

        def prefetch_x(buf, kbase):
            nc.sync.dma_start(
                out=xin[buf][:].rearrange("p (a n) -> p a n", a=ND * KT),
                in_=xarr.ap()[ds(kbase, ND * KT * 128), :]
                    .rearrange("(a p) n -> p a n", p=128))

        def chunk_body(kbase, cur):
            # scan chunk at kbase using xp[cur]; prefetch + phase A for the
            # next chunk interleaved into the scan.
            nxt = 1 - cur
            prefetch_x(nxt, kbase + ND * KT * 128)
            pa_ps = {}
            for tl in range(tc):
                if 8 <= tl and (tl - 8) // PA_SP < ND * G:
                    u_, ph = divmod(tl - 8, PA_SP)
                    if ph == 0:
                        pa_ps[u_] = emit_pa_mm(u_, xin[nxt])
                    elif ph <= PA_SUB:
                        emit_pa_bias(u_, pa_ps[u_], ph - 1, xp[nxt])
                if tl == tc // 2 + 1:
                    # first-half output flush (h rows kbase+0..255, c rows
                    # kbase+256..511) drains during the scan
                    half = (tc // 2) * B2
                    nc.sync.dma_start(
                        out=hc_out.ap()[ds(kbase, ND * 128), 0:half]
                            .rearrange("(a p) n -> p a n", p=128),
                        in_=h_hist[cur][:].rearrange(
                            "p (a n) -> p a n", a=ND)[:, :, B2:B2 + half])
                    nc.sync.dma_start(
                        out=hc_out.ap()[ds(kbase + ND * 128, ND * 128),
                                        0:half]
                            .rearrange("(a p) n -> p a n", p=128),
                        in_=c_hist[cur][:].rearrange(
                            "p (a n) -> p a n", a=ND)[:, :, B2:B2 + half])
                psg = [None, None]
                xpv = [xp[cur][d][:].rearrange("p (g t l) -> p g t l",
                                               g=G, l=BL)[:, :, tl, :]
                       for d in range(ND)]
                for d in range(ND):
                    ps = pgpool.tile([128, G * BL], f32, tag=f"g{d}",
                                     name="psg")
                    psg[d] = ps
                    # deposit xp into the gate bank ahead of the h MMs
                    if xp_dep == "dve":
                        nc.vector.tensor_copy(
                            ps[:].rearrange("p (g l) -> p g l", l=BL),
                            xpv[d])
                    else:
                        nc.tensor.matmul(
                            ps[:].rearrange("p (g l) -> p g l", l=BL),
                            ident_sb[:], xpv[d],
                            start=True, stop=False,
                            skip_group_check=True)
                    for j in range(G):
                        for ki in range(KT):
                            nc.tensor.matmul(
                                ps[:, j * BL:(j + 1) * BL],
                                w_sl(whh_sb, d, ki, j),
                                h_sl(cur, d, tl, ki, BL),
                                start=False,
                                stop=(ki == KT - 1 and j in (5, G - 1)),
                                skip_group_check=True)
                sig, amr, cf, u, tcl = [], [], [], [], []
                for d in range(ND):
                    sig.append(tpool.tile([128, G * BL], f32, tag=f"sg{d}",
                                          name="sig"))
                    amr.append(tpool.tile([128, 1], f32, tag=f"am{d}",
                                          name="amr"))
                    cf.append(tpool.tile([128, B2], f32, tag=f"cf{d}",
                                         name="cf"))
                    u.append(tpool.tile([128, B2], f32, tag=f"u{d}",
                                        name="u"))
                    tcl.append(tpool.tile([128, B2], f32, tag=f"tc{d}",
                                          name="tcl"))
                for d in range(ND):
                    # split sigmoid: [f,g,i] fires on the early stop (12 of
                    # 16 pairs); [o] is only needed much later by hmul
                    nc.scalar.activation(sig[d][:, 0:6 * BL],
                                         psg[d][:, 0:6 * BL], AF.Sigmoid)
                    nc.scalar.activation(sig[d][:, 6 * BL:8 * BL],
                                         psg[d][:, 6 * BL:8 * BL],
                                         AF.Sigmoid)
                for d in range(ND):  # cf = sig(f) * c_prev   [f tiles 0,1]
                    nc.gpsimd.tensor_mul(cf[d][:], sig[d][:, 0:2 * BL],
                                         c_sl(cur, d, tl))
                for d in range(ND):  # u = (2*sig(2g)-1) * sig(i) fused
                    nc.vector.affine_mul_reduce(
                        u[d][:], amr[d][:], sig[d][:, 2 * BL:4 * BL],
                        sig[d][:, 4 * BL:6 * BL], 2.0, -1.0)
                for d in range(ND):  # c_new (f16 history write)
                    nc.gpsimd.tensor_add(c_sl(cur, d, tl + 1),
                                         cf[d][:], u[d][:])
                for d in range(ND):
                    nc.scalar.activation(tcl[d][:], c_sl(cur, d, tl + 1),
                                         AF.Tanh)
                for d in range(ND):  # h = sig(o) * tanh(c)   [o tiles 6,7]
                    nc.vector.tensor_mul(h_sl(cur, d, tl + 1),
                                         sig[d][:, 6 * BL:8 * BL], tcl[d][:])
            # flush second-half outputs; carry state into the OTHER history
            # buffer so the drain never blocks the next chunk's scan
            half = (tc // 2) * B2
            nc.sync.dma_start(
                out=hc_out.ap()[ds(kbase, ND * 128), half:]
                    .rearrange("(a p) n -> p a n", p=128),
                in_=h_hist[cur][:].rearrange("p (a n) -> p a n",
                                             a=ND)[:, :, B2 + half:])
            nc.sync.dma_start(
                out=hc_out.ap()[ds(kbase + ND * 128, ND * 128), half:]
                    .rearrange("(a p) n -> p a n", p=128),
                in_=c_hist[cur][:].rearrange("p (a n) -> p a n",
                                             a=ND)[:, :, B2 + half:])
            for d in range(ND):
                nc.gpsimd.tensor_copy(h_sl(1 - cur, d, 0), h_sl(cur, d, tc))
                nc.gpsimd.tensor_copy(c_sl(1 - cur, d, 0), c_sl(cur, d, tc))

        # prologue: fetch + phase A for chunk 0 into buffer 0
        prefetch_x(0, 0)
        for unit in range(ND * G):
            ps = emit_pa_mm(unit, xin[0])
            for q in range(PA_SUB):
                emit_pa_bias(unit, ps, q, xp[0])

        assert n_chunks % UNROLL == 0
        CR = ND * KT * 128
        import concourse.mybir as _mb
        with tcx.For_i(0, n_chunks * CR, UNROLL * CR,
                       hint_engines=(_mb.EngineType.PE,
                                     _mb.EngineType.Activation,
                                     _mb.EngineType.DVE,
                                     _mb.EngineType.Pool)) as kbase:
            for uu in range(UNROLL):
                chunk_body(kbase + uu * CR, uu % 2)

    nc.compile()
    return nc


# ---------------- host-side data marshalling ----------------

def _perm_scale_rows(w):
    """Reorder gate rows [i,f,g,o] -> [f,g,i,o], scale g rows by 2."""
    return np.concatenate(
        [w[256:512], 2.0 * w[512:768], w[0:256], w[768:1024]], 0)


def _np_dt(dtype):
    import ml_dtypes
    return {"f32": np.float32, "bf16": ml_dtypes.bfloat16,
            "f16": np.float16}[dtype]


def prep_inputs(x, length, h0, c0, Wih_f, Whh_f, bih_f, bhh_f,
                Wih_b, Whh_b, bih_b, bhh_b, t_total=T, tc=TC, dtype=DTYPE):
    """Build per-core input dicts."""
    n_chunks = t_total // tc
    dt = _np_dt(dtype)
    x = np.asarray(x, np.float32)
    x_b = x[::-1, ::-1, :]

    wihP = {0: _perm_scale_rows(np.asarray(Wih_f)),
            1: _perm_scale_rows(np.asarray(Wih_b))}
    whhP = {0: _perm_scale_rows(np.asarray(Whh_f)),
            1: _perm_scale_rows(np.asarray(Whh_b))}
    biasP = {0: _perm_scale_rows(
                 (np.asarray(bih_f) + np.asarray(bhh_f))[:, None]),
             1: _perm_scale_rows(
                 (np.asarray(bih_b) + np.asarray(bhh_b))[:, None])}

    def wtiles(w):
        out = np.empty((ND * KT * G * 128, 128), dt)
        for d in range(ND):
            wT = w[d].T.astype(dt)
            for ki in range(KT):
                for j in range(G):
                    off = ((d * KT + ki) * G + j) * 128
                    out[off:off + 128] = wT[ki * 128:(ki + 1) * 128,
                                            j * 128:(j + 1) * 128]
        return out

    WROWS = ND * KT * G * 128
    wpack = np.empty((2 * WROWS + 128, 128), dt)
    wpack[0:WROWS] = wtiles(whhP)
    wpack[WROWS:2 * WROWS] = wtiles(wihP)
    wpack[2 * WROWS:] = np.eye(128, dtype=dt)

    biasT = np.zeros((128, ND * G), np.float32)
    for d in range(ND):
        for j in range(G):
            biasT[:, d * G + j] = biasP[d][j * 128:(j + 1) * 128, 0]

    h0 = np.asarray(h0, np.float32)
    c0 = np.asarray(c0, np.float32)

    in_maps = []
    for core in range(NCORES):
        sl = slice(core * BL, (core + 1) * BL)
        xarr = np.zeros(((n_chunks + 1) * ND * KT * 128, tc * BL), dt)
        for d, xd in ((0, x), (1, x_b)):
            xs = xd[:t_total, sl, :]
            xT = np.ascontiguousarray(xs.transpose(0, 2, 1)).astype(dt)
            for k in range(n_chunks):
                for ki in range(KT):
                    roff = (k * ND * KT + d * KT + ki) * 128
                    blk = xT[k * tc:(k + 1) * tc,
                             ki * 128:(ki + 1) * 128, :]
                    xarr[roff:roff + 128] = (
                        blk.transpose(1, 0, 2).reshape(128, tc * BL))
        hc0T = np.zeros((128, ND * 2 * B2), np.float32)
        for d in range(ND):
            for s, st in ((0, h0), (1, c0)):
                stT = st[sl].T
                for ki in range(KT):
                    off = (2 * d + s) * B2 + ki * BL
                    hc0T[:, off:off + BL] = stT[ki * 128:(ki + 1) * 128, :]
        fpack = np.concatenate([biasT, hc0T], axis=1)
        in_maps.append({"xarr": xarr, "wpack": wpack, "fpack": fpack})
    return in_maps


def assemble_outputs(results, length, t_total=T, tc=TC):
    """results: per-core {'hc_out'}. Returns (output, cell)."""
    n_chunks = t_total // tc
    length = np.asarray(length)
    out_h = np.empty((t_total, 2 * B, H), np.float32)
    out_c = np.empty((t_total, 2 * B, H), np.float32)
    for core in range(NCORES):
        sl = slice(core * BL, (core + 1) * BL)
        arr = results[core]["hc_out"]
        v = arr.astype(np.float32).reshape(n_chunks, 2, ND, 128, tc, 2, BL)
        # [k, s(h/c), d, p, tl, ki, l] -> [s, d, (k tl), l, (ki p)]
        v = v.transpose(1, 2, 0, 4, 6, 5, 3).reshape(2, ND, t_total, BL, H)
        for s, out in ((0, out_h), (1, out_c)):
            for d in range(ND):
                col0 = d * B + sl.start
                out[:, col0:col0 + BL, :] = v[s, d]
    for b in range(B):
        ln = int(length[b])
        if ln < t_total:
            out_h[ln:, b] = out_h[ln - 1, b]
            out_c[ln:, b] = out_c[ln - 1, b]
            out_h[ln:, B + b] = out_h[ln - 1, B + b]
            out_c[ln:, B + b] = out_c[ln - 1, B + b]
    return out_h, out_c


def kernel(**inputs):
    _import_bass()
    from concourse.bass_utils import run_bass_kernel_spmd
    key = (T, TC, DTYPE)
    if key not in _CACHE:
        _CACHE[key] = build_program(T, TC)
    nc = _CACHE[key]
    in_maps = prep_inputs(**inputs)
    res = run_bass_kernel_spmd(nc, in_maps, list(range(NCORES)))
    return assemble_outputs(res.results, inputs["length"])
